# revision 28
# baseline (speedup 1.0000x reference)
"""Bundle-adjustment projection kernel for 8 Trainium2 NeuronCores.

out[v, n, :] = (u, v) pixel projection of point n under view v
(reference: nn_BundleAdjustmentModel).

Sharding: data-parallel over views — 8 views per core, points replicated.

v3 design — PE-centric dense layout:
  Points processed in iterations of 16 subsets x 512 points (8192 pts).
  Block-diagonal matmuls compute dense [128, 512] tiles (partition row
  8*t + v = subset t, local view v):
     a  = (f/256)*(R0.p + tx)      (fp16 matmul -> PSUM fp32)
     b  = (f/256)*(R1.p + ty)      (fp16 matmul)
     zc = R2.p - depth             (double-fp16 matmul pair: hi product
                                    + (mh*xl + ml*xh) correction, fp32 PSUM
                                    accumulation; residual ~ 2^-22)
  Moving = points [64, 2*512] per pair of iterations (subset-major,
  x,y,z,1 on partition rows 4t..4t+3); stationaries loaded per-pair.
  Elementwise tail on full 128 partitions:
     r32 = recip(zc)  (DVE)          r16 = clip(r32, +-1e4)  (GPSIMD)
     a16 = cast(a_psum) (ACT)        b16 = cast(b_psum)      (GPSIMD)
     w_u = a16*r16 ; w_v = b16*r16   (DVE fp16 2x)
  Output w_u|w_v fp16 planes; host unscales u = cx - 256*w_u,
  v = cy + 256*w_v, reorders, interleaves.
"""
import sys
import types

import numpy as np

V = 64
N = 500000
NC = 8
NV = V // NC          # views per core
NSUB = 16             # point subsets per iteration
CW = 512              # moving free dim / PSUM bank cols (fp32)
PPI = NSUB * CW       # points per iteration = 8192
NIT = -(-N // PPI)    # 62 iterations
NPAD = NIT * PPI      # 507904
NPAIR = NIT // 2      # 31
PC = 2 * CW           # cols per pair = 1024
OCOLS = NIT * CW      # per-plane cols = 31744
AB_SCALE = 256.0
MIN_FOCAL = 50.0
MIN_DISTANCE = 0.25
Z_EPS = 1e-4

_CACHE = {}


def _setup_paths():
    if "/opt/trn_rl_repo" not in sys.path:
        sys.path.insert(0, "/opt/trn_rl_repo")
    try:
        import antenv
        if not hasattr(antenv, "axon_hooks"):
            mod = types.ModuleType("antenv.axon_hooks")
            mod._hook = None
            mod.set_axon_ntff_profile_hook = lambda h: setattr(mod, "_hook", h)
            mod.get_axon_ntff_profile_hook = lambda: mod._hook
            sys.modules["antenv.axon_hooks"] = mod
            antenv.axon_hooks = mod
    except ImportError:
        pass


def _build_nc():
    import concourse.bacc as bacc
    import concourse.mybir as mybir
    from concourse import tile

    dt = mybir.dt
    AF = mybir.ActivationFunctionType
    ALU = mybir.AluOpType

    nc = bacc.Bacc("TRN2", target_bir_lowering=False, debug=False)
    PM16 = nc.dram_tensor("PM16", [NPAIR, 64, PC], dt.float16, kind="ExternalInput")
    PM2 = nc.dram_tensor("PM2", [NPAIR, 112, PC], dt.float16, kind="ExternalInput")
    SAB = nc.dram_tensor("SAB", [64, 256], dt.float16, kind="ExternalInput")
    SZH = nc.dram_tensor("SZH", [64, 128], dt.float16, kind="ExternalInput")
    SZL = nc.dram_tensor("SZL", [112, 128], dt.float16, kind="ExternalInput")
    OUT = nc.dram_tensor("OUT", [128, 2 * OCOLS], dt.float16, kind="ExternalOutput")

    with tile.TileContext(nc) as tc:
        with (
            tc.tile_pool(name="sta", bufs=1) as sp,
            tc.tile_pool(name="mov", bufs=6) as mp,
            tc.tile_pool(name="zps", bufs=1, space="PSUM") as zp,
            tc.tile_pool(name="abps", bufs=3, space="PSUM") as abp,
            tc.tile_pool(name="stg", bufs=3) as sg,
        ):
            sab = sp.tile([64, 256], dt.float16)
            szh = sp.tile([64, 128], dt.float16)
            szl = sp.tile([112, 128], dt.float16)
            nc.sync.dma_start(out=sab[:], in_=SAB.ap())
            nc.sync.dma_start(out=szh[:], in_=SZH.ap())
            nc.sync.dma_start(out=szl[:], in_=SZL.ap())
            sa = sab[:, 0:128]
            sb = sab[:, 128:256]

            for p in range(NPAIR):
                m16 = mp.tile([64, PC], dt.float16, name="m16", tag="m16")
                m2 = mp.tile([112, PC], dt.float16, name="m2", tag="m2")
                nc.sync.dma_start(out=m16[:], in_=PM16.ap()[p])
                nc.sync.dma_start(out=m2[:], in_=PM2.ap()[p])

                r32 = sg.tile([128, PC], dt.float32, name="r32", tag="r32")
                r16 = sg.tile([128, PC], dt.float16, name="r16", tag="r16")
                ab16 = sg.tile([128, 2 * PC], dt.float16, name="ab16", tag="ab16")
                uv16 = sg.tile([128, 2 * PC], dt.float16, name="uv16", tag="uv16")

                # zc = (Szh . m16) + (Szl . m2), double-fp16 accumulation.
                # Same-stationary matmuls adjacent (hh, ll) so ldw dedup hits;
                # the two col-halves are different PSUM banks, so interleaved
                # accumulation groups are safe.
                zc2 = zp.tile([128, PC], dt.float32, name="zc2", tag="zc2")
                h_cs = [slice(h * CW, (h + 1) * CW) for h in range(2)]
                for cs in h_cs:
                    nc.tensor.matmul(out=zc2[:, cs], lhsT=szh[:], rhs=m16[:, cs],
                                     start=True, stop=False, skip_group_check=True)
                for cs in h_cs:
                    nc.tensor.matmul(out=zc2[:, cs], lhsT=szl[:], rhs=m2[:, cs],
                                     start=False, stop=True, skip_group_check=True)
                nc.vector.reciprocal_approx_fast(out=r32[:], in_=zc2[:])
                nc.gpsimd.tensor_scalar(
                    r16[:], r32[:], 1.0 / Z_EPS, -1.0 / Z_EPS, ALU.min, ALU.max)

                # a,b matmuls into per-iter a|b PSUM; single ACT cast per iter
                ab2t = [abp.tile([128, PC], dt.float32, name="ab2", tag="ab2")
                        for _ in range(2)]
                for h in range(2):
                    nc.tensor.matmul(out=ab2t[h][:, 0:CW], lhsT=sa, rhs=m16[:, h_cs[h]])
                for h in range(2):
                    nc.tensor.matmul(out=ab2t[h][:, CW:PC], lhsT=sb, rhs=m16[:, h_cs[h]])
                    nc.scalar.activation(ab16[:, h * PC:(h + 1) * PC], ab2t[h][:],
                                         AF.Copy)

                # u = a*r, v = b*r ; ab16 cols = (h, a|b, j), r16 cols = (h, j)
                abv = ab16.rearrange("q (h ab j) -> q ab h j", h=2, ab=2, j=CW)
                rv = r16.rearrange("q (h j) -> q h j", h=2)
                uvv = uv16.rearrange("q (s h j) -> q s h j", s=2, j=CW)
                nc.vector.tensor_tensor(uvv[:, 0], abv[:, 0], rv[:], ALU.mult)
                nc.vector.tensor_tensor(uvv[:, 1], abv[:, 1], rv[:], ALU.mult)
                # output DMA on the gpsimd software-DGE queue: keeps the SP
                # queue ins-only so moving-tile prefetch never blocks behind
                # an output DMA waiting on this pair's compute tail
                nc.gpsimd.dma_start(
                    out=OUT.ap()[:, p * 2 * PC:(p + 1) * 2 * PC], in_=uv16[:])
    nc.compile()
    return nc


def _host_precompute(euler, translation_xy, translation_depth_raw, focal_raw):
    """Rotations, depth, focal in fp32 numpy (replicates reference O(V) math)."""
    euler = np.asarray(euler, np.float32)
    c = np.cos(euler)
    s = np.sin(euler)
    cx_, cy_, cz_ = c[:, 0], c[:, 1], c[:, 2]
    sx_, sy_, sz_ = s[:, 0], s[:, 1], s[:, 2]
    one = np.ones_like(cx_)
    zero = np.zeros_like(cx_)
    rx = np.stack([
        np.stack([one, zero, zero], -1),
        np.stack([zero, cx_, -sx_], -1),
        np.stack([zero, sx_, cx_], -1)], -2).astype(np.float32)
    ry = np.stack([
        np.stack([cy_, zero, sy_], -1),
        np.stack([zero, one, zero], -1),
        np.stack([-sy_, zero, cy_], -1)], -2).astype(np.float32)
    rz = np.stack([
        np.stack([cz_, -sz_, zero], -1),
        np.stack([sz_, cz_, zero], -1),
        np.stack([zero, zero, one], -1)], -2).astype(np.float32)
    rot = np.matmul(np.matmul(rx, ry), rz).astype(np.float32)  # [V,3,3]

    tdr = np.asarray(translation_depth_raw, np.float32)
    depth = (np.logaddexp(tdr, np.float32(0.0)).astype(np.float32)
             + np.float32(MIN_DISTANCE)).astype(np.float32)
    fr = np.float32(np.asarray(focal_raw).reshape(-1)[0])
    focal = np.float32(np.logaddexp(fr, np.float32(0.0))) + np.float32(MIN_FOCAL)
    txy = np.asarray(translation_xy, np.float32)
    return rot, depth, focal, txy


def _block_diag(base):
    """base [4, 8] -> [64, 128] block diagonal over 16 subsets."""
    out = np.zeros((NSUB, 4, NSUB, 8), base.dtype)
    for t in range(NSUB):
        out[t, :, t, :] = base
    return out.reshape(64, 128)


def kernel(points, euler, translation_xy, translation_depth_raw, focal_raw,
           cx, cy, _trace=False):
    _setup_paths()
    from concourse.bass_utils import run_bass_kernel_spmd

    if "nc" not in _CACHE:
        _CACHE["nc"] = _build_nc()
    nc = _CACHE["nc"]

    points = np.ascontiguousarray(np.asarray(points, np.float32))
    rot, depth, focal, txy = _host_precompute(
        euler, translation_xy, translation_depth_raw, focal_raw)
    fs = focal / np.float32(AB_SCALE)

    # moving tensors, paired over iterations: [NPAIR, rows, 1024]
    pts_pad = np.zeros((NPAD, 3), np.float32)
    pts_pad[:N] = points
    arr = pts_pad.reshape(NIT, NSUB, CW, 3).transpose(0, 1, 3, 2)  # [i,t,k,j]
    xh = arr.astype(np.float16)
    xl = (arr - xh.astype(np.float32)).astype(np.float16)

    def _pair(m):  # [NIT, rows, CW] -> [NPAIR, rows, PC]
        r = m.shape[1]
        return np.ascontiguousarray(
            m.reshape(NPAIR, 2, r, CW).transpose(0, 2, 1, 3).reshape(NPAIR, r, PC))

    mov16 = np.ones((NIT, NSUB, 4, CW), np.float16)
    mov16[:, :, :3, :] = xh
    pm16 = _pair(mov16.reshape(NIT, 64, CW))
    mov2 = np.ones((NIT, NSUB, 7, CW), np.float16)
    mov2[:, :, 0:3, :] = xl
    mov2[:, :, 3:6, :] = xh
    pm2 = _pair(mov2.reshape(NIT, 112, CW))

    in_maps = []
    for core in range(NC):
        vs = slice(core * NV, (core + 1) * NV)
        r = rot[vs]           # [8,3,3]
        d = depth[vs]
        t = txy[vs]
        base_a = np.empty((4, 8), np.float32)
        base_a[:3, :] = fs * r[:, 0, :].T
        base_a[3, :] = fs * t[:, 0]
        base_b = np.empty((4, 8), np.float32)
        base_b[:3, :] = fs * r[:, 1, :].T
        base_b[3, :] = fs * t[:, 1]
        # zc double-fp16 split: mz = mzh + mzl, d = dh + dl
        mz = r[:, 2, :].T                     # [3, 8] fp32
        mzh = mz.astype(np.float16)
        mzl = (mz - mzh.astype(np.float32)).astype(np.float16)
        dh = d.astype(np.float16)
        dl = (d - dh.astype(np.float32)).astype(np.float16)
        base_zh = np.zeros((4, 8), np.float16)
        base_zh[:3, :] = mzh
        base_zh[3, :] = -dh
        base_zl = np.zeros((7, 8), np.float16)
        base_zl[0:3, :] = mzh                 # pairs with xl rows
        base_zl[3:6, :] = mzl                 # pairs with xh rows
        base_zl[6, :] = -dl
        szl = np.zeros((NSUB, 7, NSUB, 8), np.float16)
        for t in range(NSUB):
            szl[t, :, t, :] = base_zl
        sab = np.concatenate(
            [_block_diag(base_a), _block_diag(base_b)], axis=1).astype(np.float16)
        in_maps.append({
            "PM16": pm16, "PM2": pm2,
            "SAB": np.ascontiguousarray(sab),
            "SZH": np.ascontiguousarray(_block_diag(base_zh)),
            "SZL": np.ascontiguousarray(szl.reshape(112, 128)),
        })

    res = run_bass_kernel_spmd(nc, in_maps, list(range(NC)), trace=_trace)
    _CACHE["last_results"] = res

    cxf = np.float32(cx)
    cyf = np.float32(cy)
    out = np.empty((V, N, 2), np.float32)
    for core in range(NC):
        w = res.results[core]["OUT"]  # [128, 2*OCOLS] fp16
        # cols: pair p | (u,v) s | iter-half h | j  -> [128, NPAIR, 2, 2, CW]
        w = w.reshape(128, NPAIR, 2, 2, CW)
        wu = w[:, :, 0, :, :].reshape(128, OCOLS)
        wv = w[:, :, 1, :, :].reshape(128, OCOLS)
        # row 8t+v, col i*512+j  ->  view v, point (i*16+t)*512+j
        wu = wu.reshape(NSUB, NV, NIT, CW).transpose(1, 2, 0, 3).reshape(NV, NPAD)
        wv = wv.reshape(NSUB, NV, NIT, CW).transpose(1, 2, 0, 3).reshape(NV, NPAD)
        u = cxf - np.float32(AB_SCALE) * wu[:, :N].astype(np.float32)
        v = cyf + np.float32(AB_SCALE) * wv[:, :N].astype(np.float32)
        out[core * NV:(core + 1) * NV, :, 0] = u
        out[core * NV:(core + 1) * NV, :, 1] = v
    return out
